# revision 7
# baseline (speedup 1.0000x reference)
"""Spatial-softmax expectation kernel for Trainium2, 8-core SPMD.

Computes, for x of shape [B=32, C=128, H=128, W=128]:
    prob = softmax(x.reshape(B, C, H*W), axis=-1)
    keypoints[b, c] = (sum_n prob[n] * xs[w(n)], sum_n prob[n] * ys[h(n)])

Strategy (per (b, c) row the result factorizes through marginals):
    e       = exp(x - 2)                    (softmax is shift-invariant)
    marg_h  = sum_w e                       -> num_y = sum_h ys * marg_h
    marg_w  = sum_h e                       -> num_x = sum_w xs * marg_w
    s       = sum marg_h
    k       = (num_x / s, num_y / s)

Mapping: rows (b, c) on SBUF partitions, H*W on the free dim, chunked.
  - ScalarE: exp (fp32 -> fp16 output; only e is quantized, sums stay fp32)
  - VectorE: marg_h via free-axis reduce over W
  - TensorE: marg_w via fp16 identity-matmuls accumulating in fp32 PSUM
  - DMA:     ~32 MiB/core streaming, the roofline term
Sharding: data-parallel over B*C rows; core i takes rows [i*512, (i+1)*512).
"""

import numpy as np

import concourse.bacc as bacc
import concourse.mybir as mybir
import concourse.tile as tile
from concourse.bass_utils import run_bass_kernel_spmd

B, C, H, W = 32, 128, 128, 128
N_CORES = 8
ROWS = B * C                    # 4096 (b, c) rows total
ROWS_PER_CORE = ROWS // N_CORES  # 512
HW = H * W                       # 16384
GROUP = 128                      # rows per partition-group
N_GROUPS = ROWS_PER_CORE // GROUP  # 4
EXP_BIAS = -2.0                  # exp(x-2): fp16 range safety, cancels in ratio

FP32 = mybir.dt.float32
FP16 = mybir.dt.float16

_cache = {}


def build_program(
    chunk=4096,
    n_reps=1,
    loop_n=None,
    *,
    contig=True,        # map partition p -> row 4p+q (contiguous DRAM reads)
    dpool_bufs=4,
    first_split=True,   # fine sub-chunks at the stream head (faster pipe fill)
    tail_split=True,    # fine sub-chunks at the stream tail (shorter drain)
    taper_min=256,      # geometric end-taper down to this size (None: flat 512s)
    dual_ring=False,    # alternate x-loads between the two HWDGE rings
    consts_on_gpsimd=True,  # keep the x-load queue free of const DMAs
):
    """Build and compile the single-core Bass program (run SPMD on 8 cores).

    n_reps > 1 statically repeats the computation in-program; loop_n wraps it
    in a hardware For_i loop (both for benchmarking: per-rep HW time is the
    slope of wall time vs rep count, launch overhead cancels).
    """
    rows = ROWS_PER_CORE
    n_chunks = HW // chunk
    assert chunk % W == 0 and HW % chunk == 0

    nc = bacc.Bacc("TRN2", target_bir_lowering=False, debug=False)

    x_d = nc.dram_tensor("x", [rows, HW], FP32, kind="ExternalInput")
    xsb_d = nc.dram_tensor("xsb", [128, W], FP32, kind="ExternalInput")
    ysb_d = nc.dram_tensor("ysb", [128, H], FP32, kind="ExternalInput")
    idf_d = nc.dram_tensor("idf", [128, 128], FP16, kind="ExternalInput")
    y_d = nc.dram_tensor("y", [rows, 2], FP32, kind="ExternalOutput")

    def taper_sizes(total):
        """Geometric split of `total`: [total/2, total/4, ..., m, m] (m=taper_min)."""
        out, rem, s = [], total, total // 2
        while s >= taper_min:
            out.append(s)
            rem -= s
            s //= 2
        assert rem >= taper_min or rem == 0
        if rem:
            out.append(rem)
        return out

    with tile.TileContext(nc) as tc:
        with (
            tc.tile_pool(name="const", bufs=1) as cpool,
            tc.tile_pool(name="data", bufs=dpool_bufs) as dpool,
            tc.tile_pool(name="taper", bufs=3) as tpool,
            tc.tile_pool(name="marg", bufs=2) as mpool,
            tc.tile_pool(name="small", bufs=2) as spool,
            tc.tile_pool(name="psum", bufs=2, space="PSUM") as ppool,
        ):
            xsb = cpool.tile([128, W], FP32, tag="xsb")
            ysb = cpool.tile([128, H], FP32, tag="ysb")
            idf = cpool.tile([128, 128], FP16, tag="idf")
            bias_t = cpool.tile([128, 1], FP32, tag="bias")
            ceng = nc.gpsimd if consts_on_gpsimd else nc.sync
            ceng.dma_start(idf[:], idf_d[:, :])
            ceng.dma_start(xsb[:], xsb_d[:, :])
            ceng.dma_start(ysb[:], ysb_d[:, :])
            nc.vector.memset(bias_t[:], EXP_BIAS)

            if contig:
                # Partition p covers rows 4p..4p+3; group q handles row 4p+q.
                # Per-partition DMA reads are then contiguous in DRAM.
                xv = x_d[:, :].rearrange("(p q) n -> p (q n)", q=N_GROUPS)
                yv = y_d[:, :].rearrange("(p q) k -> p q k", q=N_GROUPS)

            dma_i = [0]

            def load_engine():
                if dual_ring and dma_i[0] % 2 == 1:
                    eng = nc.scalar
                else:
                    eng = nc.sync
                dma_i[0] += 1
                return eng

            def emit_group(g, first_group, last_group):
                rows_lo = g * GROUP
                margw_ps = ppool.tile([128, W], FP32, tag="margw")
                margh = mpool.tile([128, H], FP32, tag="margh")

                # Head/tail chunks sit on the exposed ends of the DMA stream
                # (head: nothing to compute until the first DMA lands; tail:
                # the last exp/reduce starts only when the stream ends), so
                # split them into fine sub-chunks to shorten the critical path.
                end_sizes = (
                    taper_sizes(chunk) if taper_min else [512] * (chunk // 512)
                )
                sizes = [chunk] * n_chunks
                if last_group and tail_split and chunk >= 1024:
                    sizes = sizes[:-1] + end_sizes
                if first_group and first_split and chunk >= 1024:
                    sizes = end_sizes[::-1] + sizes[1:]
                assert sum(sizes) == HW

                off = 0
                for size in sizes:
                    h_per = size // W
                    h_base = off // W
                    if size == chunk:
                        xt_t = dpool.tile([128, size], FP32, tag="xt")
                        et_t = dpool.tile([128, size], FP16, tag="et")
                        xt, et = xt_t[:], et_t[:]
                    else:
                        # end pieces share fixed-width pool tiles, sliced
                        xt_t = tpool.tile([128, chunk // 2], FP32, tag="xtap")
                        et_t = tpool.tile([128, chunk // 2], FP16, tag="etap")
                        xt, et = xt_t[:, :size], et_t[:, :size]
                    if contig:
                        src = xv[:, g * HW + off : g * HW + off + size]
                    else:
                        src = x_d[rows_lo : rows_lo + GROUP, off : off + size]
                    load_engine().dma_start(xt, src)
                    nc.scalar.activation(
                        et, xt, mybir.ActivationFunctionType.Exp, bias=bias_t[:]
                    )
                    e3 = et.rearrange("p (h w) -> p h w", w=W)
                    nc.vector.reduce_sum(
                        margh[:, h_base : h_base + h_per],
                        e3,
                        axis=mybir.AxisListType.X,
                    )
                    for hh in range(h_per):
                        nc.tensor.matmul(
                            margw_ps[:],
                            idf[:],
                            e3[:, hh, :],
                            start=(off == 0 and hh == 0),
                            stop=(off + size == HW and hh == h_per - 1),
                        )
                    off += size

                # tensor_tensor_reduce would fuse these, but that opcode
                # hard-faults the exec unit on this runtime; use mul+reduce.
                scr_x = spool.tile([128, W], FP32, tag="scrx")
                scr_y = spool.tile([128, H], FP32, tag="scry")
                num_xy = spool.tile([128, 2], FP32, tag="numxy")
                nc.vector.tensor_mul(scr_x[:], margw_ps[:], xsb[:])
                nc.vector.reduce_sum(num_xy[:, 0:1], scr_x[:], axis=mybir.AxisListType.X)
                nc.vector.tensor_mul(scr_y[:], margh[:], ysb[:])
                nc.vector.reduce_sum(num_xy[:, 1:2], scr_y[:], axis=mybir.AxisListType.X)
                s = spool.tile([128, 1], FP32, tag="s")
                nc.vector.reduce_sum(s[:], margh[:], axis=mybir.AxisListType.X)
                recip = spool.tile([128, 1], FP32, tag="recip")
                nc.vector.reciprocal(recip[:], s[:])
                out_t = spool.tile([128, 2], FP32, tag="out")
                nc.vector.tensor_scalar_mul(out_t[:], num_xy[:], recip[:])
                if contig:
                    dst = yv[:, g, :]
                else:
                    dst = y_d[rows_lo : rows_lo + GROUP, :]
                # HWDGE for the last group's store (lower completion latency on
                # the exposed tail); SWDGE elsewhere to keep the load FIFO clear.
                eng = nc.sync if last_group else nc.gpsimd
                eng.dma_start(dst, out_t[:])

            def emit_all():
                for _rep in range(n_reps):
                    for g in range(N_GROUPS):
                        emit_group(
                            g,
                            first_group=(g == 0),
                            last_group=(g == N_GROUPS - 1),
                        )

            if loop_n is not None:
                with tc.For_i(0, loop_n, 1, hint_engines=(mybir.EngineType.PE,)):
                    emit_all()
            else:
                emit_all()

    nc.compile()
    return nc


def make_consts():
    xs = np.linspace(-1.0, 1.0, W).astype(np.float32)
    ys = np.linspace(-1.0, 1.0, H).astype(np.float32)
    return {
        "xsb": np.ascontiguousarray(np.tile(xs, (128, 1))),
        "ysb": np.ascontiguousarray(np.tile(ys, (128, 1))),
        "idf": np.eye(128, dtype=np.float16),
    }


def kernel(x):
    x = np.ascontiguousarray(np.asarray(x), dtype=np.float32)
    assert x.shape == (B, C, H, W), x.shape

    if "nc" not in _cache:
        _cache["nc"] = build_program()
    nc = _cache["nc"]

    consts = make_consts()
    xf = x.reshape(N_CORES, ROWS_PER_CORE, HW)
    in_maps = [{"x": xf[i], **consts} for i in range(N_CORES)]
    res = run_bass_kernel_spmd(nc, in_maps, list(range(N_CORES))).results
    y = np.stack([res[i]["y"] for i in range(N_CORES)], axis=0)  # [8, 512, 2]
    return y.reshape(B, C, 2)
